# revision 55
# baseline (speedup 1.0000x reference)
"""MultiHeadDiffAttention kernel for 8 trn2 NeuronCores (v3).

Sharding: tensor-parallel over heads (H=8, one head per core).

Structure (all matmuls bf16; fp8 was tried and costs ~3.6% RMS noise per
value-path tensor — over the accuracy budget):
  - q/k projections: weight-stationary, stream xT (8 e-chunks accumulate).
  - v produced transposed (weight-stationary, 32 n=512 MMs) then flipped per
    128-chunk by PE transpose — avoids v1's LDW-bound v_group.
  - scores per k-chunk: two row-packed c=64 matmuls (both diff branches run
    concurrently in the PE array).
  - exp on ScalarE: one n=1024 ACTIVATE per k-chunk -> ee bf16.
  - u = V^T E: 2 n=512 MMs per k-chunk accumulating in PSUM.
  - softmax denominators: col-tiled partial sums - k-chunk kt accumulates into
    column-group (kt mod 4) of a shared PSUM tile, so 4 dsum matmuls run
    concurrently in the array; a tiny per-q-block fixup (copy + 2 ones-MMs)
    reduces the 4 partition-groups.
  - batch-1 projections emitted interleaved into batch-0's attention (PE
    fills ScalarE-bound gaps).
  - AllToAll carries 256-token chunks for all 8 dsts: no zero padding. Each
    core outputs 256 tokens of batch0 + 256 of batch1; host reassembles.
  - RMS applied AFTER the output projection as a per-token (=partition) scale
    in a ScalarE Copy; squares for the RMS sum run on DVE. b0's half of
    phase 3 is prefetched under batch-1 attention.
"""

import os
import sys

import numpy as np

if "/opt/trn_rl_repo" not in sys.path:
    sys.path.insert(0, "/opt/trn_rl_repo")

B, S, E, H = 2, 2048, 1024, 8
DH = E // H          # 128
F = DH // 2          # 64
P = 128              # partitions
NCORES = 8
TPC = 512            # output tokens per core (256 from each batch)
CS = 256             # a2a chunk size (tokens per dst per batch)
QBS = 512            # q-block size
QB = S // QBS        # 4 q-blocks per batch
KC = S // P          # 16 k-chunks per batch
EC = E // P          # 8 e-chunks
EPS = float(np.finfo(np.float32).eps)

LAST_RESULTS = None  # BassKernelResults of the most recent run (test.py reads this)

_NC_CACHE: dict = {}


def _build(dw: float):
    import concourse.bass as bass
    import concourse.mybir as mybir
    import concourse.tile as tile
    from concourse import bacc, masks

    dt = mybir.dt
    AF = mybir.ActivationFunctionType

    nc = bacc.Bacc("TRN2", target_bir_lowering=False, debug=False, num_devices=NCORES)

    xT_d = nc.dram_tensor("xT", [B, E, S], dt.bfloat16, kind="ExternalInput")
    wqT_d = nc.dram_tensor("wqT", [E, DH], dt.bfloat16, kind="ExternalInput")
    wkT_d = nc.dram_tensor("wkT", [E, DH], dt.bfloat16, kind="ExternalInput")
    wvT_d = nc.dram_tensor("wvT", [E, DH], dt.bfloat16, kind="ExternalInput")
    woT_d = nc.dram_tensor("woT", [E, E], dt.bfloat16, kind="ExternalInput")
    out_d = nc.dram_tensor("out", [TPC, E], dt.float32, kind="ExternalOutput")

    with tile.TileContext(nc) as tc:
        with (
            tc.tile_pool(name="consts", bufs=1) as consts,
            tc.tile_pool(name="xp", bufs=2) as xp,
            tc.tile_pool(name="qk", bufs=2) as qkp,
            tc.tile_pool(name="vp", bufs=2) as vp,
            tc.tile_pool(name="eep", bufs=5) as eep,
            tc.tile_pool(name="osb", bufs=2) as osb,
            tc.tile_pool(name="small", bufs=2) as small,
            tc.tile_pool(name="mid", bufs=2) as mid,
            tc.tile_pool(name="p3", bufs=1) as p3,
            tc.tile_pool(name="dram", bufs=1, space="DRAM") as dram,
            tc.tile_pool(name="psA", bufs=2, space="PSUM") as psA,
            tc.tile_pool(name="psU", bufs=1, space="PSUM") as psU,
            tc.tile_pool(name="psD", bufs=1, space="PSUM") as psD,
        ):
            # ---- constants ----
            eps_t = consts.tile([P, 1], dt.float32, tag="eps")
            nc.vector.memset(eps_t, EPS)
            ones_col = consts.tile([P, 32], dt.bfloat16, tag="ones_col")
            nc.vector.memset(ones_col, 1.0)
            ones_c1 = consts.tile([1, P], dt.float32, tag="ones_c1")
            nc.vector.memset(ones_c1, 1.0)
            negdw_c1 = consts.tile([1, P], dt.float32, tag="negdw_c1")
            nc.vector.memset(negdw_c1, -dw)
            idb = consts.tile([P, P], dt.bfloat16, tag="idb")
            masks.make_identity(nc, idb)
            id1 = consts.tile([1, 1], dt.bfloat16, tag="id1")
            nc.vector.memset(id1, 1.0)

            # ---- weight tiles (DMAs emitted after the first x chunk so the
            # first projection block isn't stuck behind 2MB of wo) ----
            wq_sb = consts.tile([P, EC, DH], dt.bfloat16, tag="wq")
            wk_sb = consts.tile([P, EC, DH], dt.bfloat16, tag="wk")
            wv_sb = consts.tile([P, EC, DH], dt.bfloat16, tag="wv")
            wo_sb = consts.tile([P, EC, E], dt.bfloat16, tag="wo")

            def emit_weight_dmas():
                for w_sb, w_d in ((wq_sb, wqT_d), (wk_sb, wkT_d), (wv_sb, wvT_d)):
                    nc.sync.dma_start(
                        out=w_sb, in_=w_d.rearrange("(c p) d -> p c d", p=P)
                    )

            def emit_wo_dma():
                nc.sync.dma_start(
                    out=wo_sb, in_=woT_d.rearrange("(c p) e -> p c e", p=P)
                )

            # ---- AllToAll bounce buffers: [dst, dh, 256-token chunk] ----
            a2a_in = [
                dram.tile([NCORES, DH, CS], dt.bfloat16, tag=f"a2a_in{b}",
                          name=f"a2a_in{b}")
                for b in range(B)
            ]
            a2a_out = [
                dram.tile([NCORES, DH, CS], dt.bfloat16, tag=f"a2a_out{b}",
                          name=f"a2a_out{b}")
                for b in range(B)
            ]

            # per-batch persistent tiles
            xt = [None, None]
            qT = [None, None]
            kT = [None, None]
            v = [None, None]
            oTs = None
            sq = None

            def emit_x_dma(b, tb):
                xT_v = xT_d[b].rearrange("(c p) t -> p c t", p=P)
                ts = slice(tb * QBS, (tb + 1) * QBS)
                for ec in range(EC):
                    nc.sync.dma_start(
                        out=xt[b][:, ec, ts], in_=xT_v[:, ec, ts]
                    )

            def emit_proj_block(b, tb):
                ts = slice(tb * QBS, (tb + 1) * QBS)
                psqk = psA.tile([P, 2, QBS], dt.float32, tag="sc", name="psqk")
                for ec in range(EC):
                    nc.tensor.matmul(
                        psqk[:, 0, :], lhsT=wq_sb[:, ec], rhs=xt[b][:, ec, ts],
                        start=(ec == 0), stop=(ec == EC - 1),
                    )
                for ec in range(EC):
                    nc.tensor.matmul(
                        psqk[:, 1, :], lhsT=wk_sb[:, ec], rhs=xt[b][:, ec, ts],
                        start=(ec == 0), stop=(ec == EC - 1),
                    )
                nc.vector.tensor_copy(qT[b][:, ts], psqk[:, 0, :])
                nc.vector.tensor_copy(kT[b][:, ts], psqk[:, 1, :])

                psv = psU.tile([P, 2, QBS], dt.float32, tag="u", name="psv")
                for ec in range(EC):
                    nc.tensor.matmul(
                        psv[:, 0, :], lhsT=wv_sb[:, ec], rhs=xt[b][:, ec, ts],
                        start=(ec == 0), stop=(ec == EC - 1),
                    )
                vTb = vp.tile([P, QBS], dt.bfloat16, tag="vTb")
                nc.vector.tensor_copy(vTb, psv[:, 0, :])

                # flip vT -> v for the 4 k-chunks of this block (PE transpose)
                tp4 = psD.tile([P, 4, P], dt.bfloat16, tag="d", name="tp4")
                for i in range(4):
                    nc.tensor.transpose(tp4[:, i, :], vTb[:, i * P:(i + 1) * P], idb)
                nc.vector.tensor_copy(v[b][:, 4 * tb:4 * tb + 4, :], tp4)

            def emit_attention_qb(b, qb, post_hook=None):
                qs = slice(qb * QBS, (qb + 1) * QBS)
                u12 = psU.tile([P, 2, QBS], dt.float32, tag="u", name="u12")
                dsum = psD.tile([P, 2, QBS], dt.float32, tag="d", name="dsum")

                # dsum before u: the denominator-stop gates the combine chain
                def consume_d(kt, ee):
                    for br in range(2):
                        nc.tensor.matmul(
                            dsum[0:32, br, :],
                            lhsT=ones_col, rhs=ee[:, br, :],
                            start=(kt == 0), stop=(kt == KC - 1),
                        )

                def consume_u(kt, ee):
                    for br in range(2):
                        nc.tensor.matmul(
                            u12[:, br, :], lhsT=v[b][:, kt, :], rhs=ee[:, br, :],
                            start=(kt == 0), stop=(kt == KC - 1),
                        )

                def consume(kt, ee):
                    consume_d(kt, ee)
                    consume_u(kt, ee)

                # lag-2 software pipeline: consume k-chunk kt-2 while kt's
                # scores/exp are in flight, so the u/dsum matmuls' ee operand
                # is always two ACTIVATEs old and the PE never head-of-line
                # blocks on the ScalarE exp (measured ~1us waits at lag-1)
                pend = []
                for kt in range(KC):
                    ks = slice(kt * P, (kt + 1) * P)
                    s12 = psA.tile([P, 2, QBS], dt.float32, tag="sc", name="s12")
                    nc.tensor.matmul(
                        s12[:, 0, :], lhsT=kT[b][0:F, ks], rhs=qT[b][0:F, qs]
                    )
                    nc.tensor.matmul(
                        s12[:, 1, :], lhsT=kT[b][F:P, ks], rhs=qT[b][F:P, qs]
                    )
                    ee = eep.tile([P, 2, QBS], dt.bfloat16, tag="ee")
                    nc.scalar.activation(ee, s12, AF.Exp, scale=F ** -0.5)
                    pend.append((kt, ee))
                    if len(pend) > 3:
                        consume(*pend.pop(0))
                # drain: remaining dsums first so the combine starts while the
                # last u matmuls stream
                for it in pend:
                    consume_d(*it)
                for it in pend:
                    consume_u(*it)

                # combine without touching the PE, split per branch so the
                # three engines pipeline: ScalarE lifts branch br's
                # denominator row out of PSUM (branch2 pre-scaled by 1/dw)
                # while GpSimd broadcasts the previous one and DVE takes
                # reciprocals / applies them.
                tb_ = [None, None]
                for br in range(2):
                    drow_b = small.tile([1, QBS], dt.float32, tag=f"drow{br}",
                                        name=f"drow{br}")
                    nc.scalar.activation(
                        drow_b, dsum[0:1, br, :], AF.Copy,
                        scale=(1.0 if br == 0 else 1.0 / dw),
                    )
                    dbc_b = mid.tile([P, QBS], dt.float32, tag=f"dbc{br}",
                                     name=f"dbc{br}")
                    nc.gpsimd.partition_broadcast(dbc_b, drow_b)
                    rr_b = mid.tile([P, QBS], dt.float32, tag=f"rr{br}",
                                    name=f"rr{br}")
                    nc.vector.reciprocal_approx_fast(rr_b, dbc_b)
                    t_b = mid.tile([P, QBS], dt.float32, tag=f"t{br}",
                                   name=f"t{br}")
                    nc.vector.tensor_mul(t_b, u12[:, br, :], rr_b)
                    tb_[br] = t_b
                oT = osb.tile([P, QBS], dt.bfloat16, tag="oT")
                # o = u1/d1 - u2*(dw/d2)
                nc.vector.tensor_sub(oT, tb_[0], tb_[1])
                # scatter the q-block to its two dst chunks
                nc.sync.dma_start(out=a2a_in[b][2 * qb], in_=oT[:, 0:CS])
                nc.sync.dma_start(out=a2a_in[b][2 * qb + 1], in_=oT[:, CS:QBS])
                if post_hook is not None:
                    post_hook()

            # ================= batch 0 =================
            xt[0] = xp.tile([P, EC, S], dt.bfloat16, tag="xt", name="xt0")
            qT[0] = qkp.tile([P, S], dt.bfloat16, tag="qT", name="qT0")
            kT[0] = qkp.tile([P, S], dt.bfloat16, tag="kT", name="kT0")
            v[0] = vp.tile([P, KC, DH], dt.bfloat16, tag="v", name="v0")
            emit_x_dma(0, 0)
            emit_weight_dmas()
            for tb in range(1, 4):
                emit_x_dma(0, tb)
            emit_wo_dma()
            # keep the PE's activity monitor warm while the first x/weight
            # DMAs land, so projections start at full clock
            warm_ps = psD.tile([P, 2, QBS], dt.float32, tag="d", name="warm_ps")
            for i in range(80):
                nc.tensor.matmul(
                    warm_ps[0:32, 0, 0:128], lhsT=ones_col, rhs=idb
                )
            for tb in range(4):
                emit_proj_block(0, tb)

            # batch-1 tiles + x DMA (runs during b0 attention)
            xt[1] = xp.tile([P, EC, S], dt.bfloat16, tag="xt", name="xt1")
            qT[1] = qkp.tile([P, S], dt.bfloat16, tag="qT", name="qT1")
            kT[1] = qkp.tile([P, S], dt.bfloat16, tag="kT", name="kT1")
            v[1] = vp.tile([P, KC, DH], dt.bfloat16, tag="v", name="v1")
            for tb in range(4):
                emit_x_dma(1, tb)

            for qb in range(QB):
                emit_attention_qb(0, qb)

            nc.gpsimd.collective_compute(
                "AllToAll",
                mybir.AluOpType.bypass,
                replica_groups=[list(range(NCORES))],
                ins=[a2a_in[0].opt()],
                outs=[a2a_out[0].opt()],
            )

            # batch-1 projections emitted here so they execute under A2A#1
            for tb in range(4):
                emit_proj_block(1, tb)

            # ================= batch 1 =================
            def b1_hook():
                # phase-3 prefetch: b0 half of o and its square (DVE)
                nonlocal oTs, sq
                oTs = p3.tile([P, H, TPC], dt.bfloat16, tag="oTs")
                sq = p3.tile([P, H, TPC], dt.bfloat16, tag="sq")
                nc.sync.dma_start(
                    out=oTs[:, :, 0:CS],
                    in_=a2a_out[0].rearrange("h p t -> p h t"),
                )
                nc.vector.tensor_mul(
                    sq[:, :, 0:CS], oTs[:, :, 0:CS], oTs[:, :, 0:CS]
                )

            for qb in range(QB):
                emit_attention_qb(1, qb, post_hook=b1_hook if qb == 0 else None)

            # small PE warm-keeper bridging the collective trigger window
            warm2 = psD.tile([P, 2, QBS], dt.float32, tag="d", name="warm2")
            for i in range(8):
                nc.tensor.matmul(
                    warm2[:, 0, :], lhsT=wo_sb[:, 0, 0:128], rhs=wo_sb[:, 1, 0:512]
                )

            nc.gpsimd.collective_compute(
                "AllToAll",
                mybir.AluOpType.bypass,
                replica_groups=[list(range(NCORES))],
                ins=[a2a_in[1].opt()],
                outs=[a2a_out[1].opt()],
            )

            # ---- phase 3: joint-head RMS + output projection, split by token
            # half. RMS is per-token, so the b0 half (delivered by A2A#1,
            # already on-chip) runs in full DURING the second collective; the
            # b1 half follows when A2A#2 lands. ----
            rmsT = small.tile([P, 4], dt.float32, tag="rmsT")
            out_v = out_d.rearrange("(q p) e -> q p e", p=P)

            def emit_phase3_half(c):
                cs = slice(c * CS, (c + 1) * CS)
                if c == 1:
                    # per-head contiguous DMAs (faster than one strided gather)
                    for h in range(H):
                        nc.sync.dma_start(out=oTs[:, h, cs], in_=a2a_out[1][h])
                    nc.vector.tensor_mul(sq[:, :, cs], oTs[:, :, cs],
                                         oTs[:, :, cs])
                ssq_c = psD.tile([P, CS], dt.float32, tag="d", name="ssq_c")
                for h in range(H):
                    nc.tensor.matmul(
                        ssq_c[0:32, :], lhsT=ones_col, rhs=sq[:, h, cs],
                        start=(h == 0), stop=(h == H - 1),
                    )
                sroot_c = small.tile([1, CS], dt.float32, tag="sroot")
                nc.scalar.activation(
                    sroot_c, ssq_c[0:1, :], AF.Sqrt, scale=1.0 / E,
                    bias=eps_t[0:1, :],
                )
                rmsrow_c = small.tile([1, CS], dt.float32, tag="rmsrow")
                nc.vector.reciprocal_approx_fast(rmsrow_c, sroot_c)
                rms_bf_c = small.tile([1, CS], dt.bfloat16, tag="rms_bf")
                nc.vector.tensor_copy(rms_bf_c, rmsrow_c)

                # flip rms to token-on-partition: 2 tiny PE transposes
                # (must be emitted before the copy-out ACTs that read rmsT —
                # Tile dependencies follow program order)
                rT_c = psD.tile([P, 2, 2], dt.bfloat16, tag="d", name="rT_c")
                for j in range(2):
                    nc.tensor.transpose(
                        rT_c[:, j, 0:1], rms_bf_c[0:1, j * P:(j + 1) * P], id1
                    )
                nc.vector.tensor_copy(rmsT[:, 2 * c:2 * c + 2], rT_c[:, :, 0])

                for tt in (2 * c, 2 * c + 1):
                    tts = slice(tt * P, (tt + 1) * P)
                    ps_out = psA.tile([P, 2, QBS], dt.float32, tag="sc",
                                      name="ps_out")
                    out_sb = p3.tile([P, E], dt.float32, tag="out_sb", bufs=2)
                    for nb in range(2):
                        for h in range(H):
                            nc.tensor.matmul(
                                ps_out[:, nb, :],
                                lhsT=oTs[:, h, tts],
                                rhs=wo_sb[:, h, nb * QBS:(nb + 1) * QBS],
                                start=(h == 0), stop=(h == H - 1),
                            )
                        # RMS applied as per-token (partition) scale on copy-out
                        nc.scalar.activation(
                            out_sb[:, nb * QBS:(nb + 1) * QBS], ps_out[:, nb, :],
                            AF.Copy, scale=rmsT[:, tt:tt + 1],
                        )
                    nc.sync.dma_start(out=out_v[tt], in_=out_sb)

            emit_phase3_half(0)
            # keep the PE warm through the remainder of the collective so the
            # b1-half runs at full clock when its data lands
            warm3 = psU.tile([P, 2, QBS], dt.float32, tag="u", name="warm3")
            for i in range(20):
                nc.tensor.matmul(
                    warm3[:, 0, :], lhsT=wo_sb[:, 2, 0:128], rhs=wo_sb[:, 3, 0:512]
                )
            emit_phase3_half(1)

    nc.compile()
    return nc


def _get_nc(dw: float):
    key = round(float(dw), 9)
    if key not in _NC_CACHE:
        _NC_CACHE[key] = _build(float(dw))
    return _NC_CACHE[key]


def kernel(x, Wq, Wk, Wv, norm_w, Wo, bo, diff_weight):
    import ml_dtypes

    from concourse.bass_utils import run_bass_kernel_spmd

    global LAST_RESULTS

    bf16 = ml_dtypes.bfloat16
    x = np.asarray(x, dtype=np.float32)
    Wq = np.asarray(Wq, dtype=np.float32)
    Wk = np.asarray(Wk, dtype=np.float32)
    Wv = np.asarray(Wv, dtype=np.float32)
    Wo = np.asarray(Wo, dtype=np.float32)
    norm_w = np.asarray(norm_w, dtype=np.float32)
    bo = np.asarray(bo, dtype=np.float32)
    dw = float(np.asarray(diff_weight))

    nc = _get_nc(dw)

    xT = np.ascontiguousarray(x.transpose(0, 2, 1)).astype(bf16)  # [B, E, S]
    woT = np.ascontiguousarray(
        (Wo * norm_w.reshape(-1)[None, :] * (1.0 - dw)).T
    ).astype(bf16)  # [E(feat), E(out)]

    in_maps = []
    for h in range(NCORES):
        rows = slice(h * DH, (h + 1) * DH)
        in_maps.append(
            {
                "xT": xT,
                "wqT": np.ascontiguousarray(Wq[rows, :].T).astype(bf16),
                "wkT": np.ascontiguousarray(Wk[rows, :].T).astype(bf16),
                "wvT": np.ascontiguousarray(Wv[rows, :].T).astype(bf16),
                "woT": woT,
            }
        )

    res = run_bass_kernel_spmd(
        nc,
        in_maps,
        core_ids=list(range(NCORES)),
        trace=bool(os.environ.get("KERNEL_TRACE")),
    )
    LAST_RESULTS = res

    # core c returns [512, E]: rows 0:256 = batch0 tokens [c*256,(c+1)*256),
    # rows 256:512 = batch1 same range
    full = np.empty((B, S, E), dtype=np.float32)
    for c in range(NCORES):
        o = res.results[c]["out"]
        full[0, c * CS:(c + 1) * CS] = o[0:CS]
        full[1, c * CS:(c + 1) * CS] = o[CS:TPC]
    full = full + (1.0 - dw) * bo[None, None, :]
    return full


if __name__ == "__main__":
    rng = np.random.default_rng(0)
    sc = E ** -0.5
    ins = {
        "x": rng.standard_normal((B, S, E), dtype=np.float32),
        "Wq": rng.standard_normal((E, E), dtype=np.float32) * sc,
        "Wk": rng.standard_normal((E, E), dtype=np.float32) * sc,
        "Wv": rng.standard_normal((E, E), dtype=np.float32) * sc,
        "norm_w": np.ones((H, DH), dtype=np.float32),
        "Wo": rng.standard_normal((E, E), dtype=np.float32) * sc,
        "bo": np.zeros((E,), dtype=np.float32),
        "diff_weight": np.float32(0.2),
    }
    out = kernel(**ins)
    print("out", out.shape, out.dtype, float(np.abs(out).max()))


# revision 56
# speedup vs baseline: 1.1820x; 1.1820x over previous
"""MultiHeadDiffAttention kernel for 8 trn2 NeuronCores (v3).

Sharding: tensor-parallel over heads (H=8, one head per core).

Structure (all matmuls bf16; fp8 was tried and costs ~3.6% RMS noise per
value-path tensor — over the accuracy budget):
  - q/k projections: weight-stationary, stream xT (8 e-chunks accumulate).
  - v produced transposed (weight-stationary, 32 n=512 MMs) then flipped per
    128-chunk by PE transpose — avoids v1's LDW-bound v_group.
  - scores per k-chunk: two row-packed c=64 matmuls (both diff branches run
    concurrently in the PE array).
  - exp on ScalarE: one n=1024 ACTIVATE per k-chunk -> ee bf16.
  - u = V^T E: 2 n=512 MMs per k-chunk accumulating in PSUM.
  - softmax denominators: col-tiled partial sums - k-chunk kt accumulates into
    column-group (kt mod 4) of a shared PSUM tile, so 4 dsum matmuls run
    concurrently in the array; a tiny per-q-block fixup (copy + 2 ones-MMs)
    reduces the 4 partition-groups.
  - batch-1 projections emitted interleaved into batch-0's attention (PE
    fills ScalarE-bound gaps).
  - AllToAll carries 256-token chunks for all 8 dsts: no zero padding. Each
    core outputs 256 tokens of batch0 + 256 of batch1; host reassembles.
  - RMS applied AFTER the output projection as a per-token (=partition) scale
    in a ScalarE Copy; squares for the RMS sum run on DVE. b0's half of
    phase 3 is prefetched under batch-1 attention.
"""

import os
import sys

import numpy as np

if "/opt/trn_rl_repo" not in sys.path:
    sys.path.insert(0, "/opt/trn_rl_repo")

B, S, E, H = 2, 2048, 1024, 8
DH = E // H          # 128
F = DH // 2          # 64
P = 128              # partitions
NCORES = 8
TPC = 512            # output tokens per core (256 from each batch)
CS = 256             # a2a chunk size (tokens per dst per batch)
QBS = 512            # q-block size
QB = S // QBS        # 4 q-blocks per batch
KC = S // P          # 16 k-chunks per batch
EC = E // P          # 8 e-chunks
EPS = float(np.finfo(np.float32).eps)

LAST_RESULTS = None  # BassKernelResults of the most recent run (test.py reads this)

_NC_CACHE: dict = {}


def _build(dw: float):
    import concourse.bass as bass
    import concourse.mybir as mybir
    import concourse.tile as tile
    from concourse import bacc, masks

    dt = mybir.dt
    AF = mybir.ActivationFunctionType

    nc = bacc.Bacc("TRN2", target_bir_lowering=False, debug=False, num_devices=NCORES)

    xT_d = nc.dram_tensor("xT", [B, E, S], dt.bfloat16, kind="ExternalInput")
    wqT_d = nc.dram_tensor("wqT", [E, DH], dt.bfloat16, kind="ExternalInput")
    wkT_d = nc.dram_tensor("wkT", [E, DH], dt.bfloat16, kind="ExternalInput")
    wvT_d = nc.dram_tensor("wvT", [E, DH], dt.bfloat16, kind="ExternalInput")
    woT_d = nc.dram_tensor("woT", [E, E], dt.bfloat16, kind="ExternalInput")
    out_d = nc.dram_tensor("out", [TPC, E], dt.float32, kind="ExternalOutput")

    with tile.TileContext(nc) as tc:
        with (
            tc.tile_pool(name="consts", bufs=1) as consts,
            tc.tile_pool(name="xp", bufs=2) as xp,
            tc.tile_pool(name="qk", bufs=2) as qkp,
            tc.tile_pool(name="vp", bufs=2) as vp,
            tc.tile_pool(name="eep", bufs=6) as eep,
            tc.tile_pool(name="osb", bufs=2) as osb,
            tc.tile_pool(name="small", bufs=2) as small,
            tc.tile_pool(name="mid", bufs=2) as mid,
            tc.tile_pool(name="p3", bufs=1) as p3,
            tc.tile_pool(name="dram", bufs=1, space="DRAM") as dram,
            tc.tile_pool(name="psA", bufs=2, space="PSUM") as psA,
            tc.tile_pool(name="psU", bufs=1, space="PSUM") as psU,
            tc.tile_pool(name="psD", bufs=1, space="PSUM") as psD,
        ):
            # ---- constants ----
            eps_t = consts.tile([P, 1], dt.float32, tag="eps")
            nc.vector.memset(eps_t, EPS)
            ones_col = consts.tile([P, 32], dt.bfloat16, tag="ones_col")
            nc.vector.memset(ones_col, 1.0)
            ones_c1 = consts.tile([1, P], dt.float32, tag="ones_c1")
            nc.vector.memset(ones_c1, 1.0)
            negdw_c1 = consts.tile([1, P], dt.float32, tag="negdw_c1")
            nc.vector.memset(negdw_c1, -dw)
            idb = consts.tile([P, P], dt.bfloat16, tag="idb")
            masks.make_identity(nc, idb)
            id1 = consts.tile([1, 1], dt.bfloat16, tag="id1")
            nc.vector.memset(id1, 1.0)

            # ---- weight tiles (DMAs emitted after the first x chunk so the
            # first projection block isn't stuck behind 2MB of wo) ----
            wq_sb = consts.tile([P, EC, DH], dt.bfloat16, tag="wq")
            wk_sb = consts.tile([P, EC, DH], dt.bfloat16, tag="wk")
            wv_sb = consts.tile([P, EC, DH], dt.bfloat16, tag="wv")
            wo_sb = consts.tile([P, EC, E], dt.bfloat16, tag="wo")

            def emit_weight_dmas():
                for w_sb, w_d in ((wq_sb, wqT_d), (wk_sb, wkT_d), (wv_sb, wvT_d)):
                    nc.sync.dma_start(
                        out=w_sb, in_=w_d.rearrange("(c p) d -> p c d", p=P)
                    )

            def emit_wo_dma():
                nc.sync.dma_start(
                    out=wo_sb, in_=woT_d.rearrange("(c p) e -> p c e", p=P)
                )

            # ---- AllToAll bounce buffers: [dst, dh, 256-token chunk] ----
            a2a_in = [
                dram.tile([NCORES, DH, CS], dt.bfloat16, tag=f"a2a_in{b}",
                          name=f"a2a_in{b}")
                for b in range(B)
            ]
            a2a_out = [
                dram.tile([NCORES, DH, CS], dt.bfloat16, tag=f"a2a_out{b}",
                          name=f"a2a_out{b}")
                for b in range(B)
            ]

            # per-batch persistent tiles
            xt = [None, None]
            qT = [None, None]
            kT = [None, None]
            v = [None, None]
            oTs = None
            sq = None

            def emit_x_dma(b, tb):
                xT_v = xT_d[b].rearrange("(c p) t -> p c t", p=P)
                ts = slice(tb * QBS, (tb + 1) * QBS)
                for ec in range(EC):
                    eng = nc.sync if ec % 2 == 0 else nc.gpsimd
                    eng.dma_start(out=xt[b][:, ec, ts], in_=xT_v[:, ec, ts])

            def emit_proj_block(b, tb):
                ts = slice(tb * QBS, (tb + 1) * QBS)
                psqk = psA.tile([P, 2, QBS], dt.float32, tag="sc", name="psqk")
                for ec in range(EC):
                    nc.tensor.matmul(
                        psqk[:, 0, :], lhsT=wq_sb[:, ec], rhs=xt[b][:, ec, ts],
                        start=(ec == 0), stop=(ec == EC - 1),
                    )
                for ec in range(EC):
                    nc.tensor.matmul(
                        psqk[:, 1, :], lhsT=wk_sb[:, ec], rhs=xt[b][:, ec, ts],
                        start=(ec == 0), stop=(ec == EC - 1),
                    )
                nc.vector.tensor_copy(qT[b][:, ts], psqk[:, 0, :])
                nc.vector.tensor_copy(kT[b][:, ts], psqk[:, 1, :])

                psv = psU.tile([P, 2, QBS], dt.float32, tag="u", name="psv")
                for ec in range(EC):
                    nc.tensor.matmul(
                        psv[:, 0, :], lhsT=wv_sb[:, ec], rhs=xt[b][:, ec, ts],
                        start=(ec == 0), stop=(ec == EC - 1),
                    )
                vTb = vp.tile([P, QBS], dt.bfloat16, tag="vTb")
                nc.vector.tensor_copy(vTb, psv[:, 0, :])

                # flip vT -> v for the 4 k-chunks of this block (PE transpose)
                tp4 = psD.tile([P, 4, P], dt.bfloat16, tag="d", name="tp4")
                for i in range(4):
                    nc.tensor.transpose(tp4[:, i, :], vTb[:, i * P:(i + 1) * P], idb)
                nc.vector.tensor_copy(v[b][:, 4 * tb:4 * tb + 4, :], tp4)

            def emit_attention_qb(b, qb, post_hook=None):
                qs = slice(qb * QBS, (qb + 1) * QBS)
                u12 = psU.tile([P, 2, QBS], dt.float32, tag="u", name="u12")
                dsum = psD.tile([P, 2, QBS], dt.float32, tag="d", name="dsum")

                # dsum before u: the denominator-stop gates the combine chain
                def consume_d(kt, ee):
                    for br in range(2):
                        nc.tensor.matmul(
                            dsum[0:32, br, :],
                            lhsT=ones_col, rhs=ee[:, br, :],
                            start=(kt == 0), stop=(kt == KC - 1),
                        )

                def consume_u(kt, ee):
                    for br in range(2):
                        nc.tensor.matmul(
                            u12[:, br, :], lhsT=v[b][:, kt, :], rhs=ee[:, br, :],
                            start=(kt == 0), stop=(kt == KC - 1),
                        )

                def consume(kt, ee):
                    consume_d(kt, ee)
                    consume_u(kt, ee)

                # lag-2 software pipeline: consume k-chunk kt-2 while kt's
                # scores/exp are in flight, so the u/dsum matmuls' ee operand
                # is always two ACTIVATEs old and the PE never head-of-line
                # blocks on the ScalarE exp (measured ~1us waits at lag-1)
                pend = []
                for kt in range(KC):
                    ks = slice(kt * P, (kt + 1) * P)
                    s12 = psA.tile([P, 2, QBS], dt.float32, tag="sc", name="s12")
                    nc.tensor.matmul(
                        s12[:, 0, :], lhsT=kT[b][0:F, ks], rhs=qT[b][0:F, qs]
                    )
                    nc.tensor.matmul(
                        s12[:, 1, :], lhsT=kT[b][F:P, ks], rhs=qT[b][F:P, qs]
                    )
                    ee = eep.tile([P, 2, QBS], dt.bfloat16, tag="ee")
                    nc.scalar.activation(ee, s12, AF.Exp, scale=F ** -0.5)
                    pend.append((kt, ee))
                    if len(pend) > 4:
                        consume(*pend.pop(0))
                # drain: remaining dsums first so the combine starts while the
                # last u matmuls stream
                for it in pend:
                    consume_d(*it)
                for it in pend:
                    consume_u(*it)

                # combine without touching the PE, split per branch so the
                # three engines pipeline: ScalarE lifts branch br's
                # denominator row out of PSUM (branch2 pre-scaled by 1/dw)
                # while GpSimd broadcasts the previous one and DVE takes
                # reciprocals / applies them.
                tb_ = [None, None]
                for br in range(2):
                    drow_b = small.tile([1, QBS], dt.float32, tag=f"drow{br}",
                                        name=f"drow{br}")
                    nc.scalar.activation(
                        drow_b, dsum[0:1, br, :], AF.Copy,
                        scale=(1.0 if br == 0 else 1.0 / dw),
                    )
                    dbc_b = mid.tile([P, QBS], dt.float32, tag=f"dbc{br}",
                                     name=f"dbc{br}")
                    nc.gpsimd.partition_broadcast(dbc_b, drow_b)
                    rr_b = mid.tile([P, QBS], dt.float32, tag=f"rr{br}",
                                    name=f"rr{br}")
                    nc.vector.reciprocal_approx_fast(rr_b, dbc_b)
                    t_b = mid.tile([P, QBS], dt.float32, tag=f"t{br}",
                                   name=f"t{br}")
                    nc.vector.tensor_mul(t_b, u12[:, br, :], rr_b)
                    tb_[br] = t_b
                oT = osb.tile([P, QBS], dt.bfloat16, tag="oT")
                # o = u1/d1 - u2*(dw/d2)
                nc.vector.tensor_sub(oT, tb_[0], tb_[1])
                # scatter the q-block to its two dst chunks
                nc.sync.dma_start(out=a2a_in[b][2 * qb], in_=oT[:, 0:CS])
                nc.sync.dma_start(out=a2a_in[b][2 * qb + 1], in_=oT[:, CS:QBS])
                if post_hook is not None:
                    post_hook()

            # ================= batch 0 =================
            xt[0] = xp.tile([P, EC, S], dt.bfloat16, tag="xt", name="xt0")
            qT[0] = qkp.tile([P, S], dt.bfloat16, tag="qT", name="qT0")
            kT[0] = qkp.tile([P, S], dt.bfloat16, tag="kT", name="kT0")
            v[0] = vp.tile([P, KC, DH], dt.bfloat16, tag="v", name="v0")
            emit_x_dma(0, 0)
            emit_weight_dmas()
            for tb in range(1, 4):
                emit_x_dma(0, tb)
            emit_wo_dma()
            # keep the PE's activity monitor warm while the first x/weight
            # DMAs land, so projections start at full clock
            warm_ps = psD.tile([P, 2, QBS], dt.float32, tag="d", name="warm_ps")
            for i in range(80):
                nc.tensor.matmul(
                    warm_ps[0:32, 0, 0:128], lhsT=ones_col, rhs=idb
                )
            for tb in range(4):
                emit_proj_block(0, tb)

            # batch-1 tiles + x DMA (runs during b0 attention)
            xt[1] = xp.tile([P, EC, S], dt.bfloat16, tag="xt", name="xt1")
            qT[1] = qkp.tile([P, S], dt.bfloat16, tag="qT", name="qT1")
            kT[1] = qkp.tile([P, S], dt.bfloat16, tag="kT", name="kT1")
            v[1] = vp.tile([P, KC, DH], dt.bfloat16, tag="v", name="v1")
            for tb in range(4):
                emit_x_dma(1, tb)

            for qb in range(QB):
                emit_attention_qb(0, qb)

            nc.gpsimd.collective_compute(
                "AllToAll",
                mybir.AluOpType.bypass,
                replica_groups=[list(range(NCORES))],
                ins=[a2a_in[0].opt()],
                outs=[a2a_out[0].opt()],
            )

            # batch-1 projections emitted here so they execute under A2A#1
            for tb in range(4):
                emit_proj_block(1, tb)

            # ================= batch 1 =================
            def b1_hook():
                # phase-3 prefetch: b0 half of o and its square (DVE)
                nonlocal oTs, sq
                oTs = p3.tile([P, H, TPC], dt.bfloat16, tag="oTs")
                sq = p3.tile([P, H, TPC], dt.bfloat16, tag="sq")
                nc.sync.dma_start(
                    out=oTs[:, :, 0:CS],
                    in_=a2a_out[0].rearrange("h p t -> p h t"),
                )
                nc.vector.tensor_mul(
                    sq[:, :, 0:CS], oTs[:, :, 0:CS], oTs[:, :, 0:CS]
                )

            for qb in range(QB):
                emit_attention_qb(1, qb, post_hook=b1_hook if qb == 0 else None)

            # small PE warm-keeper bridging the collective trigger window
            warm2 = psD.tile([P, 2, QBS], dt.float32, tag="d", name="warm2")
            for i in range(8):
                nc.tensor.matmul(
                    warm2[:, 0, :], lhsT=wo_sb[:, 0, 0:128], rhs=wo_sb[:, 1, 0:512]
                )

            nc.gpsimd.collective_compute(
                "AllToAll",
                mybir.AluOpType.bypass,
                replica_groups=[list(range(NCORES))],
                ins=[a2a_in[1].opt()],
                outs=[a2a_out[1].opt()],
            )

            # ---- phase 3: joint-head RMS + output projection, split by token
            # half. RMS is per-token, so the b0 half (delivered by A2A#1,
            # already on-chip) runs in full DURING the second collective; the
            # b1 half follows when A2A#2 lands. ----
            rmsT = small.tile([P, 4], dt.float32, tag="rmsT")
            out_v = out_d.rearrange("(q p) e -> q p e", p=P)

            def emit_phase3_half(c):
                cs = slice(c * CS, (c + 1) * CS)
                if c == 1:
                    # per-head contiguous DMAs (faster than one strided gather)
                    for h in range(H):
                        nc.sync.dma_start(out=oTs[:, h, cs], in_=a2a_out[1][h])
                    nc.vector.tensor_mul(sq[:, :, cs], oTs[:, :, cs],
                                         oTs[:, :, cs])
                ssq_c = psD.tile([P, CS], dt.float32, tag="d", name="ssq_c")
                for h in range(H):
                    nc.tensor.matmul(
                        ssq_c[0:32, :], lhsT=ones_col, rhs=sq[:, h, cs],
                        start=(h == 0), stop=(h == H - 1),
                    )
                sroot_c = small.tile([1, CS], dt.float32, tag="sroot")
                nc.scalar.activation(
                    sroot_c, ssq_c[0:1, :], AF.Sqrt, scale=1.0 / E,
                    bias=eps_t[0:1, :],
                )
                rmsrow_c = small.tile([1, CS], dt.float32, tag="rmsrow")
                nc.vector.reciprocal_approx_fast(rmsrow_c, sroot_c)
                rms_bf_c = small.tile([1, CS], dt.bfloat16, tag="rms_bf")
                nc.vector.tensor_copy(rms_bf_c, rmsrow_c)

                # flip rms to token-on-partition: 2 tiny PE transposes
                # (must be emitted before the copy-out ACTs that read rmsT —
                # Tile dependencies follow program order)
                rT_c = psD.tile([P, 2, 2], dt.bfloat16, tag="d", name="rT_c")
                for j in range(2):
                    nc.tensor.transpose(
                        rT_c[:, j, 0:1], rms_bf_c[0:1, j * P:(j + 1) * P], id1
                    )
                nc.vector.tensor_copy(rmsT[:, 2 * c:2 * c + 2], rT_c[:, :, 0])

                for tt in (2 * c, 2 * c + 1):
                    tts = slice(tt * P, (tt + 1) * P)
                    ps_out = psA.tile([P, 2, QBS], dt.float32, tag="sc",
                                      name="ps_out")
                    out_sb = p3.tile([P, E], dt.float32, tag="out_sb", bufs=2)
                    for nb in range(2):
                        for h in range(H):
                            nc.tensor.matmul(
                                ps_out[:, nb, :],
                                lhsT=oTs[:, h, tts],
                                rhs=wo_sb[:, h, nb * QBS:(nb + 1) * QBS],
                                start=(h == 0), stop=(h == H - 1),
                            )
                        # RMS applied as per-token (partition) scale on copy-out
                        nc.scalar.activation(
                            out_sb[:, nb * QBS:(nb + 1) * QBS], ps_out[:, nb, :],
                            AF.Copy, scale=rmsT[:, tt:tt + 1],
                        )
                    nc.sync.dma_start(out=out_v[tt], in_=out_sb)

            emit_phase3_half(0)
            # keep the PE warm through the remainder of the collective so the
            # b1-half runs at full clock when its data lands
            warm3 = psU.tile([P, 2, QBS], dt.float32, tag="u", name="warm3")
            for i in range(32):
                nc.tensor.matmul(
                    warm3[:, 0, :], lhsT=wo_sb[:, 2, 0:128], rhs=wo_sb[:, 3, 0:512]
                )
            emit_phase3_half(1)

    nc.compile()
    return nc


def _get_nc(dw: float):
    key = round(float(dw), 9)
    if key not in _NC_CACHE:
        _NC_CACHE[key] = _build(float(dw))
    return _NC_CACHE[key]


def kernel(x, Wq, Wk, Wv, norm_w, Wo, bo, diff_weight):
    import ml_dtypes

    from concourse.bass_utils import run_bass_kernel_spmd

    global LAST_RESULTS

    bf16 = ml_dtypes.bfloat16
    x = np.asarray(x, dtype=np.float32)
    Wq = np.asarray(Wq, dtype=np.float32)
    Wk = np.asarray(Wk, dtype=np.float32)
    Wv = np.asarray(Wv, dtype=np.float32)
    Wo = np.asarray(Wo, dtype=np.float32)
    norm_w = np.asarray(norm_w, dtype=np.float32)
    bo = np.asarray(bo, dtype=np.float32)
    dw = float(np.asarray(diff_weight))

    nc = _get_nc(dw)

    xT = np.ascontiguousarray(x.transpose(0, 2, 1)).astype(bf16)  # [B, E, S]
    woT = np.ascontiguousarray(
        (Wo * norm_w.reshape(-1)[None, :] * (1.0 - dw)).T
    ).astype(bf16)  # [E(feat), E(out)]

    in_maps = []
    for h in range(NCORES):
        rows = slice(h * DH, (h + 1) * DH)
        in_maps.append(
            {
                "xT": xT,
                "wqT": np.ascontiguousarray(Wq[rows, :].T).astype(bf16),
                "wkT": np.ascontiguousarray(Wk[rows, :].T).astype(bf16),
                "wvT": np.ascontiguousarray(Wv[rows, :].T).astype(bf16),
                "woT": woT,
            }
        )

    res = run_bass_kernel_spmd(
        nc,
        in_maps,
        core_ids=list(range(NCORES)),
        trace=bool(os.environ.get("KERNEL_TRACE")),
    )
    LAST_RESULTS = res

    # core c returns [512, E]: rows 0:256 = batch0 tokens [c*256,(c+1)*256),
    # rows 256:512 = batch1 same range
    full = np.empty((B, S, E), dtype=np.float32)
    for c in range(NCORES):
        o = res.results[c]["out"]
        full[0, c * CS:(c + 1) * CS] = o[0:CS]
        full[1, c * CS:(c + 1) * CS] = o[CS:TPC]
    full = full + (1.0 - dw) * bo[None, None, :]
    return full


if __name__ == "__main__":
    rng = np.random.default_rng(0)
    sc = E ** -0.5
    ins = {
        "x": rng.standard_normal((B, S, E), dtype=np.float32),
        "Wq": rng.standard_normal((E, E), dtype=np.float32) * sc,
        "Wk": rng.standard_normal((E, E), dtype=np.float32) * sc,
        "Wv": rng.standard_normal((E, E), dtype=np.float32) * sc,
        "norm_w": np.ones((H, DH), dtype=np.float32),
        "Wo": rng.standard_normal((E, E), dtype=np.float32) * sc,
        "bo": np.zeros((E,), dtype=np.float32),
        "diff_weight": np.float32(0.2),
    }
    out = kernel(**ins)
    print("out", out.shape, out.dtype, float(np.abs(out).max()))


# revision 58
# speedup vs baseline: 1.1917x; 1.0082x over previous
"""MultiHeadDiffAttention kernel for 8 trn2 NeuronCores (v3).

Sharding: tensor-parallel over heads (H=8, one head per core).

Structure (all matmuls bf16; fp8 was tried and costs ~3.6% RMS noise per
value-path tensor — over the accuracy budget):
  - q/k projections: weight-stationary, stream xT (8 e-chunks accumulate).
  - v produced transposed (weight-stationary, 32 n=512 MMs) then flipped per
    128-chunk by PE transpose — avoids v1's LDW-bound v_group.
  - scores per k-chunk: two row-packed c=64 matmuls (both diff branches run
    concurrently in the PE array).
  - exp on ScalarE: one n=1024 ACTIVATE per k-chunk -> ee bf16.
  - u = V^T E: 2 n=512 MMs per k-chunk accumulating in PSUM.
  - softmax denominators: col-tiled partial sums - k-chunk kt accumulates into
    column-group (kt mod 4) of a shared PSUM tile, so 4 dsum matmuls run
    concurrently in the array; a tiny per-q-block fixup (copy + 2 ones-MMs)
    reduces the 4 partition-groups.
  - batch-1 projections emitted interleaved into batch-0's attention (PE
    fills ScalarE-bound gaps).
  - AllToAll carries 256-token chunks for all 8 dsts: no zero padding. Each
    core outputs 256 tokens of batch0 + 256 of batch1; host reassembles.
  - RMS applied AFTER the output projection as a per-token (=partition) scale
    in a ScalarE Copy; squares for the RMS sum run on DVE. b0's half of
    phase 3 is prefetched under batch-1 attention.
"""

import os
import sys

import numpy as np

if "/opt/trn_rl_repo" not in sys.path:
    sys.path.insert(0, "/opt/trn_rl_repo")

B, S, E, H = 2, 2048, 1024, 8
DH = E // H          # 128
F = DH // 2          # 64
P = 128              # partitions
NCORES = 8
TPC = 512            # output tokens per core (256 from each batch)
CS = 256             # a2a chunk size (tokens per dst per batch)
QBS = 512            # q-block size
QB = S // QBS        # 4 q-blocks per batch
KC = S // P          # 16 k-chunks per batch
EC = E // P          # 8 e-chunks
EPS = float(np.finfo(np.float32).eps)

LAST_RESULTS = None  # BassKernelResults of the most recent run (test.py reads this)

_NC_CACHE: dict = {}


def _build(dw: float):
    import concourse.bass as bass
    import concourse.mybir as mybir
    import concourse.tile as tile
    from concourse import bacc, masks

    dt = mybir.dt
    AF = mybir.ActivationFunctionType

    nc = bacc.Bacc("TRN2", target_bir_lowering=False, debug=False, num_devices=NCORES)

    xT_d = nc.dram_tensor("xT", [B, E, S], dt.bfloat16, kind="ExternalInput")
    wqT_d = nc.dram_tensor("wqT", [E, DH], dt.bfloat16, kind="ExternalInput")
    wkT_d = nc.dram_tensor("wkT", [E, DH], dt.bfloat16, kind="ExternalInput")
    wvT_d = nc.dram_tensor("wvT", [E, DH], dt.bfloat16, kind="ExternalInput")
    woT_d = nc.dram_tensor("woT", [E, E], dt.bfloat16, kind="ExternalInput")
    out_d = nc.dram_tensor("out", [TPC, E], dt.float32, kind="ExternalOutput")

    with tile.TileContext(nc) as tc:
        with (
            tc.tile_pool(name="consts", bufs=1) as consts,
            tc.tile_pool(name="xp", bufs=2) as xp,
            tc.tile_pool(name="qk", bufs=2) as qkp,
            tc.tile_pool(name="vp", bufs=2) as vp,
            tc.tile_pool(name="eep", bufs=6) as eep,
            tc.tile_pool(name="osb", bufs=2) as osb,
            tc.tile_pool(name="small", bufs=2) as small,
            tc.tile_pool(name="mid", bufs=2) as mid,
            tc.tile_pool(name="p3", bufs=1) as p3,
            tc.tile_pool(name="dram", bufs=1, space="DRAM") as dram,
            tc.tile_pool(name="psA", bufs=2, space="PSUM") as psA,
            tc.tile_pool(name="psU", bufs=1, space="PSUM") as psU,
            tc.tile_pool(name="psD", bufs=1, space="PSUM") as psD,
        ):
            # ---- constants ----
            eps_t = consts.tile([P, 1], dt.float32, tag="eps")
            nc.vector.memset(eps_t, EPS)
            ones_col = consts.tile([P, 32], dt.bfloat16, tag="ones_col")
            nc.vector.memset(ones_col, 1.0)
            ones_c1 = consts.tile([1, P], dt.float32, tag="ones_c1")
            nc.vector.memset(ones_c1, 1.0)
            negdw_c1 = consts.tile([1, P], dt.float32, tag="negdw_c1")
            nc.vector.memset(negdw_c1, -dw)
            idb = consts.tile([P, P], dt.bfloat16, tag="idb")
            masks.make_identity(nc, idb)
            id1 = consts.tile([1, 1], dt.bfloat16, tag="id1")
            nc.vector.memset(id1, 1.0)

            # ---- weight tiles (DMAs emitted after the first x chunk so the
            # first projection block isn't stuck behind 2MB of wo) ----
            wq_sb = consts.tile([P, EC, DH], dt.bfloat16, tag="wq")
            wk_sb = consts.tile([P, EC, DH], dt.bfloat16, tag="wk")
            wv_sb = consts.tile([P, EC, DH], dt.bfloat16, tag="wv")
            wo_sb = consts.tile([P, EC, E], dt.bfloat16, tag="wo")

            def emit_weight_dmas():
                # third queue: land in parallel with the x chunks at startup
                for w_sb, w_d in ((wq_sb, wqT_d), (wk_sb, wkT_d), (wv_sb, wvT_d)):
                    nc.scalar.dma_start(
                        out=w_sb, in_=w_d.rearrange("(c p) d -> p c d", p=P)
                    )

            def emit_wo_dma():
                nc.sync.dma_start(
                    out=wo_sb, in_=woT_d.rearrange("(c p) e -> p c e", p=P)
                )

            # ---- AllToAll bounce buffers: [dst, dh, 256-token chunk] ----
            a2a_in = [
                dram.tile([NCORES, DH, CS], dt.bfloat16, tag=f"a2a_in{b}",
                          name=f"a2a_in{b}")
                for b in range(B)
            ]
            a2a_out = [
                dram.tile([NCORES, DH, CS], dt.bfloat16, tag=f"a2a_out{b}",
                          name=f"a2a_out{b}")
                for b in range(B)
            ]

            # per-batch persistent tiles
            xt = [None, None]
            qT = [None, None]
            kT = [None, None]
            v = [None, None]
            oTs = None
            sq = None

            def emit_x_dma(b, tb):
                xT_v = xT_d[b].rearrange("(c p) t -> p c t", p=P)
                ts = slice(tb * QBS, (tb + 1) * QBS)
                for ec in range(EC):
                    eng = nc.sync if ec % 2 == 0 else nc.gpsimd
                    eng.dma_start(out=xt[b][:, ec, ts], in_=xT_v[:, ec, ts])

            def emit_proj_block(b, tb):
                ts = slice(tb * QBS, (tb + 1) * QBS)
                psqk = psA.tile([P, 2, QBS], dt.float32, tag="sc", name="psqk")
                for ec in range(EC):
                    nc.tensor.matmul(
                        psqk[:, 0, :], lhsT=wq_sb[:, ec], rhs=xt[b][:, ec, ts],
                        start=(ec == 0), stop=(ec == EC - 1),
                    )
                for ec in range(EC):
                    nc.tensor.matmul(
                        psqk[:, 1, :], lhsT=wk_sb[:, ec], rhs=xt[b][:, ec, ts],
                        start=(ec == 0), stop=(ec == EC - 1),
                    )
                nc.vector.tensor_copy(qT[b][:, ts], psqk[:, 0, :])
                nc.vector.tensor_copy(kT[b][:, ts], psqk[:, 1, :])

                psv = psU.tile([P, 2, QBS], dt.float32, tag="u", name="psv")
                for ec in range(EC):
                    nc.tensor.matmul(
                        psv[:, 0, :], lhsT=wv_sb[:, ec], rhs=xt[b][:, ec, ts],
                        start=(ec == 0), stop=(ec == EC - 1),
                    )
                vTb = vp.tile([P, QBS], dt.bfloat16, tag="vTb")
                nc.vector.tensor_copy(vTb, psv[:, 0, :])

                # flip vT -> v for the 4 k-chunks of this block (PE transpose)
                tp4 = psD.tile([P, 4, P], dt.bfloat16, tag="d", name="tp4")
                for i in range(4):
                    nc.tensor.transpose(tp4[:, i, :], vTb[:, i * P:(i + 1) * P], idb)
                nc.vector.tensor_copy(v[b][:, 4 * tb:4 * tb + 4, :], tp4)

            def emit_attention_qb(b, qb, post_hook=None):
                qs = slice(qb * QBS, (qb + 1) * QBS)
                u12 = psU.tile([P, 2, QBS], dt.float32, tag="u", name="u12")
                dsum = psD.tile([P, 2, QBS], dt.float32, tag="d", name="dsum")

                # dsum before u: the denominator-stop gates the combine chain
                def consume_d(kt, ee):
                    for br in range(2):
                        nc.tensor.matmul(
                            dsum[0:32, br, :],
                            lhsT=ones_col, rhs=ee[:, br, :],
                            start=(kt == 0), stop=(kt == KC - 1),
                        )

                def consume_u(kt, ee):
                    for br in range(2):
                        nc.tensor.matmul(
                            u12[:, br, :], lhsT=v[b][:, kt, :], rhs=ee[:, br, :],
                            start=(kt == 0), stop=(kt == KC - 1),
                        )

                def consume(kt, ee):
                    consume_d(kt, ee)
                    consume_u(kt, ee)

                # lag-2 software pipeline: consume k-chunk kt-2 while kt's
                # scores/exp are in flight, so the u/dsum matmuls' ee operand
                # is always two ACTIVATEs old and the PE never head-of-line
                # blocks on the ScalarE exp (measured ~1us waits at lag-1)
                pend = []
                for kt in range(KC):
                    ks = slice(kt * P, (kt + 1) * P)
                    s12 = psA.tile([P, 2, QBS], dt.float32, tag="sc", name="s12")
                    nc.tensor.matmul(
                        s12[:, 0, :], lhsT=kT[b][0:F, ks], rhs=qT[b][0:F, qs]
                    )
                    nc.tensor.matmul(
                        s12[:, 1, :], lhsT=kT[b][F:P, ks], rhs=qT[b][F:P, qs]
                    )
                    ee = eep.tile([P, 2, QBS], dt.bfloat16, tag="ee")
                    nc.scalar.activation(ee, s12, AF.Exp, scale=F ** -0.5)
                    pend.append((kt, ee))
                    if len(pend) > 4:
                        consume(*pend.pop(0))
                # drain: remaining dsums first so the combine starts while the
                # last u matmuls stream
                for it in pend:
                    consume_d(*it)
                for it in pend:
                    consume_u(*it)

                # combine without touching the PE, split per branch so the
                # three engines pipeline: ScalarE lifts branch br's
                # denominator row out of PSUM (branch2 pre-scaled by 1/dw)
                # while GpSimd broadcasts the previous one and DVE takes
                # reciprocals / applies them.
                tb_ = [None, None]
                for br in range(2):
                    drow_b = small.tile([1, QBS], dt.float32, tag=f"drow{br}",
                                        name=f"drow{br}")
                    nc.scalar.activation(
                        drow_b, dsum[0:1, br, :], AF.Copy,
                        scale=(1.0 if br == 0 else 1.0 / dw),
                    )
                    dbc_b = mid.tile([P, QBS], dt.float32, tag=f"dbc{br}",
                                     name=f"dbc{br}")
                    nc.gpsimd.partition_broadcast(dbc_b, drow_b)
                    rr_b = mid.tile([P, QBS], dt.float32, tag=f"rr{br}",
                                    name=f"rr{br}")
                    nc.vector.reciprocal_approx_fast(rr_b, dbc_b)
                    t_b = mid.tile([P, QBS], dt.float32, tag=f"t{br}",
                                   name=f"t{br}")
                    nc.vector.tensor_mul(t_b, u12[:, br, :], rr_b)
                    tb_[br] = t_b
                oT = osb.tile([P, QBS], dt.bfloat16, tag="oT")
                # o = u1/d1 - u2*(dw/d2)
                nc.vector.tensor_sub(oT, tb_[0], tb_[1])
                # scatter the q-block to its two dst chunks
                nc.sync.dma_start(out=a2a_in[b][2 * qb], in_=oT[:, 0:CS])
                nc.sync.dma_start(out=a2a_in[b][2 * qb + 1], in_=oT[:, CS:QBS])
                if post_hook is not None:
                    post_hook()

            # ================= batch 0 =================
            xt[0] = xp.tile([P, EC, S], dt.bfloat16, tag="xt", name="xt0")
            qT[0] = qkp.tile([P, S], dt.bfloat16, tag="qT", name="qT0")
            kT[0] = qkp.tile([P, S], dt.bfloat16, tag="kT", name="kT0")
            v[0] = vp.tile([P, KC, DH], dt.bfloat16, tag="v", name="v0")
            emit_x_dma(0, 0)
            emit_weight_dmas()
            for tb in range(1, 4):
                emit_x_dma(0, tb)
            emit_wo_dma()
            # keep the PE's activity monitor warm while the first x/weight
            # DMAs land, so projections start at full clock
            warm_ps = psD.tile([P, 2, QBS], dt.float32, tag="d", name="warm_ps")
            for i in range(80):
                nc.tensor.matmul(
                    warm_ps[0:32, 0, 0:128], lhsT=ones_col, rhs=idb
                )
            for tb in range(4):
                emit_proj_block(0, tb)

            # batch-1 tiles + x DMA (runs during b0 attention)
            xt[1] = xp.tile([P, EC, S], dt.bfloat16, tag="xt", name="xt1")
            qT[1] = qkp.tile([P, S], dt.bfloat16, tag="qT", name="qT1")
            kT[1] = qkp.tile([P, S], dt.bfloat16, tag="kT", name="kT1")
            v[1] = vp.tile([P, KC, DH], dt.bfloat16, tag="v", name="v1")
            for tb in range(4):
                emit_x_dma(1, tb)

            for qb in range(QB):
                emit_attention_qb(0, qb)

            nc.gpsimd.collective_compute(
                "AllToAll",
                mybir.AluOpType.bypass,
                replica_groups=[list(range(NCORES))],
                ins=[a2a_in[0].opt()],
                outs=[a2a_out[0].opt()],
            )

            # batch-1 projections emitted here so they execute under A2A#1
            for tb in range(4):
                emit_proj_block(1, tb)

            # ================= batch 1 =================
            def b1_hook():
                # phase-3 prefetch: b0 half of o and its square (DVE)
                nonlocal oTs, sq
                oTs = p3.tile([P, H, TPC], dt.bfloat16, tag="oTs")
                sq = p3.tile([P, H, TPC], dt.bfloat16, tag="sq")
                nc.sync.dma_start(
                    out=oTs[:, :, 0:CS],
                    in_=a2a_out[0].rearrange("h p t -> p h t"),
                )
                nc.vector.tensor_mul(
                    sq[:, :, 0:CS], oTs[:, :, 0:CS], oTs[:, :, 0:CS]
                )

            for qb in range(QB):
                emit_attention_qb(1, qb, post_hook=b1_hook if qb == 0 else None)

            # small PE warm-keeper bridging the collective trigger window
            warm2 = psD.tile([P, 2, QBS], dt.float32, tag="d", name="warm2")
            for i in range(8):
                nc.tensor.matmul(
                    warm2[:, 0, :], lhsT=wo_sb[:, 0, 0:128], rhs=wo_sb[:, 1, 0:512]
                )

            nc.gpsimd.collective_compute(
                "AllToAll",
                mybir.AluOpType.bypass,
                replica_groups=[list(range(NCORES))],
                ins=[a2a_in[1].opt()],
                outs=[a2a_out[1].opt()],
            )

            # ---- phase 3: joint-head RMS + output projection, split by token
            # half. RMS is per-token, so the b0 half (delivered by A2A#1,
            # already on-chip) runs in full DURING the second collective; the
            # b1 half follows when A2A#2 lands. ----
            rmsT = small.tile([P, 4], dt.float32, tag="rmsT")
            out_v = out_d.rearrange("(q p) e -> q p e", p=P)

            def emit_phase3_half(c):
                cs = slice(c * CS, (c + 1) * CS)
                if c == 1:
                    # per-head contiguous DMAs (faster than one strided gather)
                    for h in range(H):
                        nc.sync.dma_start(out=oTs[:, h, cs], in_=a2a_out[1][h])
                    nc.vector.tensor_mul(sq[:, :, cs], oTs[:, :, cs],
                                         oTs[:, :, cs])
                ssq_c = psD.tile([P, CS], dt.float32, tag="d", name="ssq_c")
                for h in range(H):
                    nc.tensor.matmul(
                        ssq_c[0:32, :], lhsT=ones_col, rhs=sq[:, h, cs],
                        start=(h == 0), stop=(h == H - 1),
                    )
                sroot_c = small.tile([1, CS], dt.float32, tag="sroot")
                nc.scalar.activation(
                    sroot_c, ssq_c[0:1, :], AF.Sqrt, scale=1.0 / E,
                    bias=eps_t[0:1, :],
                )
                rmsrow_c = small.tile([1, CS], dt.float32, tag="rmsrow")
                nc.vector.reciprocal_approx_fast(rmsrow_c, sroot_c)
                rms_bf_c = small.tile([1, CS], dt.bfloat16, tag="rms_bf")
                nc.vector.tensor_copy(rms_bf_c, rmsrow_c)

                # flip rms to token-on-partition: 2 tiny PE transposes
                # (must be emitted before the copy-out ACTs that read rmsT —
                # Tile dependencies follow program order)
                rT_c = psD.tile([P, 2, 2], dt.bfloat16, tag="d", name="rT_c")
                for j in range(2):
                    nc.tensor.transpose(
                        rT_c[:, j, 0:1], rms_bf_c[0:1, j * P:(j + 1) * P], id1
                    )
                nc.vector.tensor_copy(rmsT[:, 2 * c:2 * c + 2], rT_c[:, :, 0])

                for tt in (2 * c, 2 * c + 1):
                    tts = slice(tt * P, (tt + 1) * P)
                    ps_out = psA.tile([P, 2, QBS], dt.float32, tag="sc",
                                      name="ps_out")
                    out_sb = p3.tile([P, E], dt.float32, tag="out_sb", bufs=2)
                    for nb in range(2):
                        for h in range(H):
                            nc.tensor.matmul(
                                ps_out[:, nb, :],
                                lhsT=oTs[:, h, tts],
                                rhs=wo_sb[:, h, nb * QBS:(nb + 1) * QBS],
                                start=(h == 0), stop=(h == H - 1),
                            )
                        # RMS applied as per-token (partition) scale on copy-out
                        nc.scalar.activation(
                            out_sb[:, nb * QBS:(nb + 1) * QBS], ps_out[:, nb, :],
                            AF.Copy, scale=rmsT[:, tt:tt + 1],
                        )
                    nc.sync.dma_start(out=out_v[tt], in_=out_sb)

            emit_phase3_half(0)
            # keep the PE warm through the remainder of the collective so the
            # b1-half runs at full clock when its data lands
            warm3 = psU.tile([P, 2, QBS], dt.float32, tag="u", name="warm3")
            for i in range(32):
                nc.tensor.matmul(
                    warm3[:, 0, :], lhsT=wo_sb[:, 2, 0:128], rhs=wo_sb[:, 3, 0:512]
                )
            emit_phase3_half(1)

    nc.compile()
    return nc


def _get_nc(dw: float):
    key = round(float(dw), 9)
    if key not in _NC_CACHE:
        _NC_CACHE[key] = _build(float(dw))
    return _NC_CACHE[key]


def kernel(x, Wq, Wk, Wv, norm_w, Wo, bo, diff_weight):
    import ml_dtypes

    from concourse.bass_utils import run_bass_kernel_spmd

    global LAST_RESULTS

    bf16 = ml_dtypes.bfloat16
    x = np.asarray(x, dtype=np.float32)
    Wq = np.asarray(Wq, dtype=np.float32)
    Wk = np.asarray(Wk, dtype=np.float32)
    Wv = np.asarray(Wv, dtype=np.float32)
    Wo = np.asarray(Wo, dtype=np.float32)
    norm_w = np.asarray(norm_w, dtype=np.float32)
    bo = np.asarray(bo, dtype=np.float32)
    dw = float(np.asarray(diff_weight))

    nc = _get_nc(dw)

    xT = np.ascontiguousarray(x.transpose(0, 2, 1)).astype(bf16)  # [B, E, S]
    woT = np.ascontiguousarray(
        (Wo * norm_w.reshape(-1)[None, :] * (1.0 - dw)).T
    ).astype(bf16)  # [E(feat), E(out)]

    in_maps = []
    for h in range(NCORES):
        rows = slice(h * DH, (h + 1) * DH)
        in_maps.append(
            {
                "xT": xT,
                "wqT": np.ascontiguousarray(Wq[rows, :].T).astype(bf16),
                "wkT": np.ascontiguousarray(Wk[rows, :].T).astype(bf16),
                "wvT": np.ascontiguousarray(Wv[rows, :].T).astype(bf16),
                "woT": woT,
            }
        )

    res = run_bass_kernel_spmd(
        nc,
        in_maps,
        core_ids=list(range(NCORES)),
        trace=bool(os.environ.get("KERNEL_TRACE")),
    )
    LAST_RESULTS = res

    # core c returns [512, E]: rows 0:256 = batch0 tokens [c*256,(c+1)*256),
    # rows 256:512 = batch1 same range
    full = np.empty((B, S, E), dtype=np.float32)
    for c in range(NCORES):
        o = res.results[c]["out"]
        full[0, c * CS:(c + 1) * CS] = o[0:CS]
        full[1, c * CS:(c + 1) * CS] = o[CS:TPC]
    full = full + (1.0 - dw) * bo[None, None, :]
    return full


if __name__ == "__main__":
    rng = np.random.default_rng(0)
    sc = E ** -0.5
    ins = {
        "x": rng.standard_normal((B, S, E), dtype=np.float32),
        "Wq": rng.standard_normal((E, E), dtype=np.float32) * sc,
        "Wk": rng.standard_normal((E, E), dtype=np.float32) * sc,
        "Wv": rng.standard_normal((E, E), dtype=np.float32) * sc,
        "norm_w": np.ones((H, DH), dtype=np.float32),
        "Wo": rng.standard_normal((E, E), dtype=np.float32) * sc,
        "bo": np.zeros((E,), dtype=np.float32),
        "diff_weight": np.float32(0.2),
    }
    out = kernel(**ins)
    print("out", out.shape, out.dtype, float(np.abs(out).max()))
